# revision 21
# baseline (speedup 1.0000x reference)
"""AttnDecoderRNN single-step decoder on 8 Trainium2 NeuronCores.

Tensor-parallel with only TWO collectives on the serial chain:
  A) attn logits slice [512] = attn_in @ attn_W_k.T  (attn_W/enc L-sharded)
     w = exp(logit + b); u_partial = w @ enc_k; S via ones-block column
     -> AllGather [u(1024)|S(x128)] + local sum  (collective #1, 4.6KB)
  B) x = relu(comb_W @ [emb; u/S] + b)  -- comb_W REPLICATED so every core
     has the full x (no collective)
  C) LSTM gate slices (4H g-sharded) -> c1,h1 slices locally (no collective)
  D) out_W sharded along the CONTRACTION (h) dim: core k needs only its own
     h1 slice -> partial logits for the FULL padded vocab [51200]
     -> AllReduce logits (collective #2, 205KB) -> local log-softmax ->
     every core writes full logp; the host slices per-core ranges.
Big weights are bf16 (fp32 PSUM accumulation), host pre-tiled so every DMA
is contiguous per partition. The embedding row is gathered on the host
(4KB of the 206MB table). Vocab padded to 51200 with -1e30 bias (exp->0).
"""

import sys

if "/opt/trn_rl_repo" not in sys.path:
    sys.path.insert(0, "/opt/trn_rl_repo")

import numpy as np
import ml_dtypes

P = 128
I, H, L, V = 1024, 1024, 4096, 50257
NCORES = 8
LK = L // NCORES            # 512 attn rows / core
MC_A = LK // P              # 4
KC_IH = (I + H) // P        # 16 contraction chunks over 2048
KC_H = H // P               # 8
V_PAD = 51200
MC_D = V_PAD // P           # 400 vocab chunks (full padded vocab per core)
VK = V_PAD // NCORES        # 6400 output rows / core
D_GRP = 8                   # vocab chunks per psum/dma group in stage D
N_GRP = MC_D // D_GRP       # 50

F32 = np.float32
BF16 = ml_dtypes.bfloat16

_CACHE = {}

# small-tensor column map
C_AIN, C_AB, C_CB, C_BIH, C_BHH, C_C0, C_OB = 0, 16, 20, 28, 32, 36, 37
SMALL_COLS = C_OB + MC_D    # 437


def _build_program():
    import concourse.bass as bass  # noqa: F401
    import concourse.mybir as mybir
    import concourse.tile as tile
    from concourse import bacc

    dt = mybir.dt.float32
    bt = mybir.dt.bfloat16
    AF = mybir.ActivationFunctionType

    nc = bacc.Bacc("TRN2", target_bir_lowering=False, debug=False,
                   num_devices=NCORES, enable_asserts=False)

    # ---- I/O ---- (all weight layouts are partition(j)-major contiguous)
    awt_d = nc.dram_tensor("awt", [P, MC_A, KC_IH, P], bt, kind="ExternalInput")
    enc_d = nc.dram_tensor("enc", [P, MC_A, H + P], bt, kind="ExternalInput")
    cwt_d = nc.dram_tensor("cwt", [P, KC_IH, KC_H, P], bt, kind="ExternalInput")
    gwt_d = nc.dram_tensor("gwt", [P, 4, KC_IH, P], bt, kind="ExternalInput")
    owt_d = nc.dram_tensor("owt", [P, MC_D, P], bt, kind="ExternalInput")
    small_d = nc.dram_tensor("small", [P, SMALL_COLS], dt, kind="ExternalInput")

    logp_o = nc.dram_tensor("logp_o", [P * MC_D], dt, kind="ExternalOutput")
    attnw_o = nc.dram_tensor("attnw_o", [LK], dt, kind="ExternalOutput")
    h_o = nc.dram_tensor("h_o", [P], dt, kind="ExternalOutput")
    c_o = nc.dram_tensor("c_o", [P], dt, kind="ExternalOutput")

    warm_i = nc.dram_tensor("warm_i", [1], dt)
    warm_o = nc.dram_tensor("warm_o", [NCORES], dt, addr_space="Shared")
    cc1_i = nc.dram_tensor("cc1_i", [H + P], dt)
    cc1_o = nc.dram_tensor("cc1_o", [H + P], dt, addr_space="Shared")
    ar_i = nc.dram_tensor("ar_i", [P * MC_D], dt)
    ar_o = nc.dram_tensor("ar_o", [P * MC_D], dt, addr_space="Shared")
    RG = [list(range(NCORES))]

    from concourse.bass import _add_dep_helper

    with tile.TileContext(nc) as tc:
        with (
            tc.tile_pool(name="const", bufs=1) as const,
            tc.tile_pool(name="ow", bufs=N_GRP) as owp,
            tc.tile_pool(name="ps", bufs=2, space="PSUM") as ps,
            tc.tile_pool(name="psd", bufs=4, space="PSUM") as psd,
        ):
            # ncfw pays ~35us of one-time warmup on the FIRST collective;
            # absorb it under the weight loads with a dummy gather.
            nc.gpsimd.collective_compute(
                "AllGather", mybir.AluOpType.bypass, replica_groups=RG,
                ins=[warm_i.ap().opt()], outs=[warm_o.ap().opt()])

            # ---- resident weight loads ----
            small = const.tile([P, SMALL_COLS], dt)
            nc.gpsimd.dma_start(small[:], small_d[:])
            awt = const.tile([P, MC_A, KC_IH, P], bt)
            nc.sync.dma_start(awt[:], awt_d[:])
            enc = const.tile([P, MC_A, H + P], bt)
            nc.sync.dma_start(enc[:], enc_d[:])
            cwt = const.tile([P, KC_IH, KC_H, P], bt)
            cwt_dma = nc.sync.dma_start(cwt[:], cwt_d[:])
            gwt = const.tile([P, 4, KC_IH, P], bt)
            gwt_dma = nc.sync.dma_start(gwt[:], gwt_d[:])

            ones128 = const.tile([P, P], dt)
            nc.vector.memset(ones128[:], 1.0)

            ain = small[:, C_AIN:C_AIN + 16]   # [emb(8) | h0(8)] cols
            ab = small[:, C_AB:C_AB + 4]
            cb = small[:, C_CB:C_CB + 8]
            bih = small[:, C_BIH:C_BIH + 4]
            bhh = small[:, C_BHH:C_BHH + 4]
            c0 = small[:, C_C0:C_C0 + 1]
            ob = small[:, C_OB:C_OB + MC_D]
            ain_bf = const.tile([P, 16], bt)
            nc.vector.tensor_copy(ain_bf[:], ain)

            # ================= Stage A: attention =================
            wT = const.tile([P, MC_A], dt)
            wT_bf = const.tile([P, MC_A], bt)
            for mc in range(MC_A):
                pa = ps.tile([P, 1], dt, tag="ps")
                for kc in range(KC_IH):
                    nc.tensor.matmul(pa[:], awt[:, mc, kc], ain_bf[:, kc:kc + 1],
                                     start=(kc == 0), stop=(kc == KC_IH - 1))
                nc.scalar.activation(wT[:, mc:mc + 1], pa[:], AF.Exp,
                                     bias=ab[:, mc:mc + 1])
                nc.vector.tensor_copy(wT_bf[:, mc:mc + 1], wT[:, mc:mc + 1])

            # u[h] = sum_l w[l] enc[l, h]; chunk 8 hits the ones block -> S
            KA = KC_H + 1
            uT = const.tile([P, KA], dt)
            for hc in range(KA):
                pu = ps.tile([P, 1], dt, tag="ps")
                for mc in range(MC_A):
                    nc.tensor.matmul(pu[:], enc[:, mc, hc * P:(hc + 1) * P],
                                     wT_bf[:, mc:mc + 1],
                                     start=(mc == 0), stop=(mc == MC_A - 1))
                nc.vector.tensor_copy(uT[:, hc:hc + 1], pu[:])

            # CC1: AllReduce [u | S] (ncfw entry already paid by warmup)
            cc1w = nc.gpsimd.dma_start(cc1_i.rearrange("(hc p) -> p hc", p=P), uT[:])
            cc1cc = nc.gpsimd.collective_compute(
                "AllReduce", mybir.AluOpType.add, replica_groups=RG,
                ins=[cc1_i.ap().opt()], outs=[cc1_o.ap().opt()])
            # true quiet window: B/C/D weights all stream after CC1's mesh
            _add_dep_helper(cwt_dma.ins, cc1cc.ins, reason="bulk after CC1")
            _add_dep_helper(gwt_dma.ins, cc1cc.ins, reason="bulk after CC1")
            uTs = const.tile([P, KA], dt)
            nc.gpsimd.dma_start(uTs[:], cc1_o.rearrange("(hc p) -> p hc", p=P))
            sinv128 = const.tile([P, 1], dt)
            nc.vector.reciprocal(sinv128[:], uTs[:, KC_H:KA])

            wout = const.tile([P, MC_A], dt)
            nc.vector.tensor_scalar_mul(wout[:], wT[:], sinv128[:])
            nc.gpsimd.dma_start(attnw_o.rearrange("(mc p) -> p mc", p=P), wout[:])
            uA = const.tile([P, KC_H], dt)
            nc.vector.tensor_scalar_mul(uA[:], uTs[:, 0:KC_H], sinv128[:])
            uA_bf = const.tile([P, KC_H], bt)
            nc.vector.tensor_copy(uA_bf[:], uA[:])

            # ========== Stage B: combine + relu (replicated: full x) ==========
            x_bf = const.tile([P, KC_H], bt)
            for hc in range(KC_H):
                px = ps.tile([P, 1], dt, tag="ps")
                for kc in range(KC_IH):
                    rhs = ain_bf[:, kc:kc + 1] if kc < 8 else uA_bf[:, kc - 8:kc - 7]
                    nc.tensor.matmul(px[:], cwt[:, kc, hc], rhs,
                                     start=(kc == 0), stop=(kc == KC_IH - 1))
                xs = const.tile([P, 1], dt, tag="xs")
                nc.scalar.activation(xs[:], px[:], AF.Relu, bias=cb[:, hc:hc + 1])
                nc.vector.tensor_copy(x_bf[:, hc:hc + 1], xs[:])

            # ================= Stage C: LSTM step =================
            bias2 = const.tile([P, 4], dt)
            nc.vector.tensor_add(bias2[:], bih, bhh)
            gates = []
            for g in range(4):
                pg = ps.tile([P, 1], dt, tag="ps")
                for kc in range(KC_IH):
                    rhs = x_bf[:, kc:kc + 1] if kc < 8 else ain_bf[:, kc:kc + 1]
                    nc.tensor.matmul(pg[:], gwt[:, g, kc], rhs,
                                     start=(kc == 0), stop=(kc == KC_IH - 1))
                act = AF.Tanh if g == 2 else AF.Sigmoid
                gs = const.tile([P, 1], dt, tag=f"gate{g}")
                nc.scalar.activation(gs[:], pg[:], act, bias=bias2[:, g:g + 1])
                gates.append(gs)
            i_g, f_g, g_g, o_g = gates
            t1 = const.tile([P, 1], dt)
            nc.vector.tensor_mul(t1[:], f_g[:], c0)
            t2 = const.tile([P, 1], dt)
            nc.vector.tensor_mul(t2[:], i_g[:], g_g[:])
            c1 = const.tile([P, 1], dt)
            nc.vector.tensor_add(c1[:], t1[:], t2[:])
            tc1 = const.tile([P, 1], dt)
            nc.scalar.activation(tc1[:], c1[:], AF.Tanh)
            h1 = const.tile([P, 1], dt)
            nc.vector.tensor_mul(h1[:], o_g[:], tc1[:])
            nc.gpsimd.dma_start(c_o[:], c1[:])
            nc.gpsimd.dma_start(h_o[:], h1[:])
            h1_bf = const.tile([P, 1], bt)
            nc.vector.tensor_copy(h1_bf[:], h1[:])

            # ==== Stage D: partial logits for the FULL vocab (h-sharded) ====
            logits = const.tile([P, MC_D], dt)
            for g8 in range(N_GRP):
                owt_t = owp.tile([P, D_GRP, P], bt, tag="ow")
                ow_dma = nc.sync.dma_start(
                    owt_t[:], owt_d[:, g8 * D_GRP:(g8 + 1) * D_GRP, :])
                _add_dep_helper(ow_dma.ins, cc1cc.ins, reason="bulk after CC1")
                pd = psd.tile([P, D_GRP], dt, tag="pd")
                for j in range(D_GRP):
                    nc.tensor.matmul(pd[:, j:j + 1], owt_t[:, j], h1_bf[:],
                                     start=True, stop=True)
                nc.vector.tensor_copy(
                    logits[:, g8 * D_GRP:(g8 + 1) * D_GRP], pd[:])

            # CC2: AllReduce partial logits (p-major payload, 205KB)
            nc.gpsimd.dma_start(ar_i.rearrange("(p mc) -> p mc", p=P), logits[:])
            nc.gpsimd.collective_compute(
                "AllReduce", mybir.AluOpType.add, replica_groups=RG,
                ins=[ar_i.ap().opt()], outs=[ar_o.ap().opt()])
            lfull = const.tile([P, MC_D], dt)
            nc.gpsimd.dma_start(lfull[:], ar_o.rearrange("(p mc) -> p mc", p=P))
            nc.vector.tensor_add(lfull[:], lfull[:], ob)

            # local log-softmax over the full padded vocab
            exps = const.tile([P, MC_D], dt)
            nc.scalar.activation(exps[:], lfull[:], AF.Exp)
            rowsum = const.tile([P, 1], dt)
            nc.vector.reduce_sum(rowsum[:], exps[:], axis=mybir.AxisListType.X)
            psv = ps.tile([P, 1], dt, tag="ps")
            nc.tensor.matmul(psv[:], ones128[:], rowsum[:])
            sv128 = const.tile([P, 1], dt)
            nc.vector.tensor_copy(sv128[:], psv[:])
            logzb = const.tile([P, 1], dt)
            nc.scalar.activation(logzb[:], sv128[:], AF.Ln)

            # full logp out (p-major); the host slices each core's range
            logp = const.tile([P, MC_D], dt)
            nc.vector.tensor_scalar_sub(logp[:], lfull[:], logzb[:])
            nc.gpsimd.dma_start(logp_o.rearrange("(p mc) -> p mc", p=P), logp[:])

    nc.compile()
    return nc


def _prep_inputs(x):
    """Host-side shard/transpose/tile. Pure data movement + one row gather."""
    tok = int(np.asarray(x["input_token"]).reshape(-1)[0])
    emb_row = np.asarray(x["emb"])[tok].astype(F32)          # [1024]
    h0 = np.asarray(x["h"], F32).reshape(H)
    c0 = np.asarray(x["c"], F32).reshape(H)
    attn_in = np.concatenate([emb_row, h0])                   # [2048]
    attn_W = np.asarray(x["attn_W"], F32)
    attn_b = np.asarray(x["attn_b"], F32)
    enc = np.asarray(x["encoder_outputs"], F32)
    comb_W = np.asarray(x["comb_W"], F32)
    comb_b = np.asarray(x["comb_b"], F32)
    Wcat = np.concatenate([np.asarray(x["W_ih"], F32),
                           np.asarray(x["W_hh"], F32)], axis=1)  # [4096, 2048]
    b_ih = np.asarray(x["b_ih"], F32).reshape(4, NCORES, P)
    b_hh = np.asarray(x["b_hh"], F32).reshape(4, NCORES, P)
    out_W = np.asarray(x["out_W"], F32)
    out_b = np.asarray(x["out_b"], F32)
    ow_pad = np.zeros((V_PAD, H), F32)
    ow_pad[:V] = out_W
    ob_pad = np.full((V_PAD,), -1e30, F32)
    ob_pad[:V] = out_b

    ain_c = np.ascontiguousarray(attn_in.reshape(16, P).T)    # [128,16]
    # comb replicated: [j, kc, hc, p] = comb_W[hc*128+p, kc*128+j]
    cwt = np.ascontiguousarray(
        comb_W.reshape(KC_H, P, KC_IH, P).transpose(3, 2, 0, 1).astype(BF16))

    in_maps = []
    for k in range(NCORES):
        A_k = attn_W[k * LK:(k + 1) * LK]
        awt = np.ascontiguousarray(
            A_k.reshape(MC_A, P, KC_IH, P).transpose(3, 0, 2, 1).astype(BF16))
        enc_aug = np.concatenate(
            [enc[k * LK:(k + 1) * LK], np.ones((LK, P), F32)], axis=1)
        enc_t = np.ascontiguousarray(
            enc_aug.reshape(MC_A, P, H + P).transpose(1, 0, 2).astype(BF16))
        gwt = np.empty((P, 4, KC_IH, P), BF16)
        for g in range(4):
            G = Wcat[g * H + k * P: g * H + (k + 1) * P]
            gwt[:, g] = G.reshape(P, KC_IH, P).transpose(2, 1, 0).astype(BF16)
        # out_W h-sharded: [j, mc, v] = ow_pad[mc*128+v, k*128+j]
        owt = np.ascontiguousarray(
            ow_pad[:, k * P:(k + 1) * P]
            .reshape(MC_D, P, P).transpose(2, 0, 1).astype(BF16))
        small = np.zeros((P, SMALL_COLS), F32)
        small[:, C_AIN:C_AIN + 16] = ain_c
        small[:, C_AB:C_AB + 4] = attn_b[k * LK:(k + 1) * LK].reshape(MC_A, P).T
        small[:, C_CB:C_CB + 8] = comb_b.reshape(KC_H, P).T
        small[:, C_BIH:C_BIH + 4] = b_ih[:, k, :].T
        small[:, C_BHH:C_BHH + 4] = b_hh[:, k, :].T
        small[:, C_C0] = c0[k * P:(k + 1) * P]
        small[:, C_OB:C_OB + MC_D] = ob_pad.reshape(MC_D, P).T
        in_maps.append({
            "awt": awt, "enc": enc_t, "cwt": cwt, "gwt": gwt,
            "owt": owt, "small": np.ascontiguousarray(small),
        })
    return in_maps


def _install_ntff_hook():
    """Recreate the missing antenv.axon_hooks shim so trace=True works."""
    import types, ctypes, contextlib

    if "antenv.axon_hooks" in sys.modules:
        return
    so_path = "/opt/axon/libaxon_pjrt.so"
    lib = ctypes.CDLL(so_path)
    if not hasattr(lib, "axon_start_nrt_profile"):
        return
    lib.axon_start_nrt_profile.argtypes = [ctypes.POINTER(ctypes.c_int64),
                                           ctypes.c_size_t]
    lib.axon_start_nrt_profile.restype = ctypes.c_int64
    lib.axon_stop_nrt_profile.argtypes = [ctypes.c_char_p]
    lib.axon_stop_nrt_profile.restype = ctypes.c_int64

    @contextlib.contextmanager
    def _hook(output_dir, device_ids):
        import jax
        jax.devices()
        if device_ids:
            ids = (ctypes.c_int64 * len(device_ids))(*device_ids)
            rc = lib.axon_start_nrt_profile(ids, len(device_ids))
        else:
            rc = lib.axon_start_nrt_profile(None, 0)
        if rc != 0:
            raise RuntimeError(f"axon_start_nrt_profile rc={rc}")
        try:
            yield
        finally:
            n = lib.axon_stop_nrt_profile(str(output_dir).encode())
            print(f"ntff profile: {n} file(s) written to {output_dir}",
                  file=sys.stderr)

    mod = types.ModuleType("antenv.axon_hooks")
    mod._hook = _hook
    mod.get_axon_ntff_profile_hook = lambda: _hook
    mod.set_axon_ntff_profile_hook = lambda h: None
    sys.modules["antenv.axon_hooks"] = mod


def _run(in_maps, trace=False, **kw):
    from concourse import bass_utils
    if trace:
        _install_ntff_hook()
    if "nc" not in _CACHE:
        _CACHE["nc"] = _build_program()
    return bass_utils.run_bass_kernel_spmd(
        _CACHE["nc"], in_maps, core_ids=list(range(NCORES)), trace=trace, **kw)


def _logp_slice(res_k, k):
    full = res_k["logp_o"].reshape(P, MC_D).T.reshape(-1)   # vocab order
    return full[k * VK:(k + 1) * VK]


def kernel(**inputs):
    in_maps = _prep_inputs(inputs)
    res = _run(in_maps)
    logp = np.concatenate(
        [_logp_slice(res.results[k], k) for k in range(NCORES)])[:V].reshape(1, V)
    attnw = np.concatenate([r["attnw_o"] for r in res.results]).reshape(1, L)
    h1 = np.concatenate([r["h_o"] for r in res.results]).reshape(1, 1, H)
    c1 = np.concatenate([r["c_o"] for r in res.results]).reshape(1, 1, H)
    return logp, (h1, c1), attnw


# revision 22
# speedup vs baseline: 1.1403x; 1.1403x over previous
"""AttnDecoderRNN single-step decoder on 8 Trainium2 NeuronCores.

Tensor-parallel with only TWO collectives on the serial chain:
  A) attn logits slice [512] = attn_in @ attn_W_k.T  (attn_W/enc L-sharded)
     w = exp(logit + b); u_partial = w @ enc_k; S via ones-block column
     -> AllGather [u(1024)|S(x128)] + local sum  (collective #1, 4.6KB)
  B) x = relu(comb_W @ [emb; u/S] + b)  -- comb_W REPLICATED so every core
     has the full x (no collective)
  C) LSTM gate slices (4H g-sharded) -> c1,h1 slices locally (no collective)
  D) out_W sharded along the CONTRACTION (h) dim: core k needs only its own
     h1 slice -> partial logits for the FULL padded vocab [51200]
     -> AllReduce logits (collective #2, 205KB) -> local log-softmax ->
     every core writes full logp; the host slices per-core ranges.
Big weights are bf16 (fp32 PSUM accumulation), host pre-tiled so every DMA
is contiguous per partition. The embedding row is gathered on the host
(4KB of the 206MB table). Vocab padded to 51200 with -1e30 bias (exp->0).
"""

import sys

if "/opt/trn_rl_repo" not in sys.path:
    sys.path.insert(0, "/opt/trn_rl_repo")

import numpy as np
import ml_dtypes

P = 128
I, H, L, V = 1024, 1024, 4096, 50257
NCORES = 8
LK = L // NCORES            # 512 attn rows / core
MC_A = LK // P              # 4
KC_IH = (I + H) // P        # 16 contraction chunks over 2048
KC_H = H // P               # 8
V_PAD = 51200
MC_D = V_PAD // P           # 400 vocab chunks (full padded vocab per core)
VK = V_PAD // NCORES        # 6400 output rows / core
D_GRP = 8                   # vocab chunks per psum/dma group in stage D
N_GRP = MC_D // D_GRP       # 50

F32 = np.float32
BF16 = ml_dtypes.bfloat16

_CACHE = {}

# small-tensor column map
C_AIN, C_AB, C_CB, C_BIH, C_BHH, C_C0, C_OB = 0, 16, 20, 28, 32, 36, 37
SMALL_COLS = C_OB + MC_D    # 437


def _build_program():
    import concourse.bass as bass  # noqa: F401
    import concourse.mybir as mybir
    import concourse.tile as tile
    from concourse import bacc

    dt = mybir.dt.float32
    bt = mybir.dt.bfloat16
    AF = mybir.ActivationFunctionType

    nc = bacc.Bacc("TRN2", target_bir_lowering=False, debug=False,
                   num_devices=NCORES, enable_asserts=False)

    # ---- I/O ---- (all weight layouts are partition(j)-major contiguous)
    awt_d = nc.dram_tensor("awt", [P, MC_A, KC_IH, P], bt, kind="ExternalInput")
    enc_d = nc.dram_tensor("enc", [P, MC_A, H + P], bt, kind="ExternalInput")
    cwt_d = nc.dram_tensor("cwt", [P, KC_IH, KC_H, P], bt, kind="ExternalInput")
    gwt_d = nc.dram_tensor("gwt", [P, 4, KC_IH, P], bt, kind="ExternalInput")
    owt_d = nc.dram_tensor("owt", [P, MC_D, P], bt, kind="ExternalInput")
    small_d = nc.dram_tensor("small", [P, SMALL_COLS], dt, kind="ExternalInput")

    logp_o = nc.dram_tensor("logp_o", [P * MC_D], dt, kind="ExternalOutput")
    attnw_o = nc.dram_tensor("attnw_o", [LK], dt, kind="ExternalOutput")
    h_o = nc.dram_tensor("h_o", [P], dt, kind="ExternalOutput")
    c_o = nc.dram_tensor("c_o", [P], dt, kind="ExternalOutput")

    warm_i = nc.dram_tensor("warm_i", [1], dt)
    warm_o = nc.dram_tensor("warm_o", [1], dt, addr_space="Shared")
    cc1_i = nc.dram_tensor("cc1_i", [H + P], dt)
    cc1_o = nc.dram_tensor("cc1_o", [H + P], dt, addr_space="Shared")
    ar_i = nc.dram_tensor("ar_i", [P * MC_D], dt)
    ar_o = nc.dram_tensor("ar_o", [P * MC_D], dt, addr_space="Shared")
    RG = [list(range(NCORES))]

    from concourse.bass import _add_dep_helper

    with tile.TileContext(nc) as tc:
        with (
            tc.tile_pool(name="const", bufs=1) as const,
            tc.tile_pool(name="ow", bufs=N_GRP) as owp,
            tc.tile_pool(name="ps", bufs=2, space="PSUM") as ps,
            tc.tile_pool(name="psd", bufs=4, space="PSUM") as psd,
        ):
            # ncfw pays ~35us of one-time warmup on the FIRST collective;
            # absorb it under the weight loads with a dummy gather.
            nc.gpsimd.collective_compute(
                "AllReduce", mybir.AluOpType.add,
                replica_groups=[[r] for r in range(NCORES)],
                ins=[warm_i.ap().opt()], outs=[warm_o.ap().opt()])

            # ---- resident weight loads ----
            small = const.tile([P, SMALL_COLS], dt)
            nc.gpsimd.dma_start(small[:], small_d[:])
            awt = const.tile([P, MC_A, KC_IH, P], bt)
            nc.sync.dma_start(awt[:], awt_d[:])
            enc = const.tile([P, MC_A, H + P], bt)
            nc.sync.dma_start(enc[:], enc_d[:])
            cwt = const.tile([P, KC_IH, KC_H, P], bt)
            cwt_dma = nc.sync.dma_start(cwt[:], cwt_d[:])
            gwt = const.tile([P, 4, KC_IH, P], bt)
            gwt_dma = nc.sync.dma_start(gwt[:], gwt_d[:])

            ones128 = const.tile([P, P], dt)
            nc.vector.memset(ones128[:], 1.0)

            ain = small[:, C_AIN:C_AIN + 16]   # [emb(8) | h0(8)] cols
            ab = small[:, C_AB:C_AB + 4]
            cb = small[:, C_CB:C_CB + 8]
            bih = small[:, C_BIH:C_BIH + 4]
            bhh = small[:, C_BHH:C_BHH + 4]
            c0 = small[:, C_C0:C_C0 + 1]
            ob = small[:, C_OB:C_OB + MC_D]
            ain_bf = const.tile([P, 16], bt)
            nc.vector.tensor_copy(ain_bf[:], ain)

            # ================= Stage A: attention =================
            wT = const.tile([P, MC_A], dt)
            wT_bf = const.tile([P, MC_A], bt)
            for mc in range(MC_A):
                pa = ps.tile([P, 1], dt, tag="ps")
                for kc in range(KC_IH):
                    nc.tensor.matmul(pa[:], awt[:, mc, kc], ain_bf[:, kc:kc + 1],
                                     start=(kc == 0), stop=(kc == KC_IH - 1))
                nc.scalar.activation(wT[:, mc:mc + 1], pa[:], AF.Exp,
                                     bias=ab[:, mc:mc + 1])
                nc.vector.tensor_copy(wT_bf[:, mc:mc + 1], wT[:, mc:mc + 1])

            # u[h] = sum_l w[l] enc[l, h]; chunk 8 hits the ones block -> S
            KA = KC_H + 1
            uT = const.tile([P, KA], dt)
            for hc in range(KA):
                pu = ps.tile([P, 1], dt, tag="ps")
                for mc in range(MC_A):
                    nc.tensor.matmul(pu[:], enc[:, mc, hc * P:(hc + 1) * P],
                                     wT_bf[:, mc:mc + 1],
                                     start=(mc == 0), stop=(mc == MC_A - 1))
                nc.vector.tensor_copy(uT[:, hc:hc + 1], pu[:])

            # CC1: AllReduce [u | S] (ncfw entry already paid by warmup)
            cc1w = nc.gpsimd.dma_start(cc1_i.rearrange("(hc p) -> p hc", p=P), uT[:])
            nc.gpsimd.collective_compute(
                "AllReduce", mybir.AluOpType.add, replica_groups=RG,
                ins=[cc1_i.ap().opt()], outs=[cc1_o.ap().opt()])
            uTs = const.tile([P, KA], dt)
            nc.gpsimd.dma_start(uTs[:], cc1_o.rearrange("(hc p) -> p hc", p=P))
            sinv128 = const.tile([P, 1], dt)
            nc.vector.reciprocal(sinv128[:], uTs[:, KC_H:KA])

            wout = const.tile([P, MC_A], dt)
            nc.vector.tensor_scalar_mul(wout[:], wT[:], sinv128[:])
            nc.gpsimd.dma_start(attnw_o.rearrange("(mc p) -> p mc", p=P), wout[:])
            uA = const.tile([P, KC_H], dt)
            nc.vector.tensor_scalar_mul(uA[:], uTs[:, 0:KC_H], sinv128[:])
            uA_bf = const.tile([P, KC_H], bt)
            nc.vector.tensor_copy(uA_bf[:], uA[:])

            # ========== Stage B: combine + relu (replicated: full x) ==========
            x_bf = const.tile([P, KC_H], bt)
            for hc in range(KC_H):
                px = ps.tile([P, 1], dt, tag="ps")
                for kc in range(KC_IH):
                    rhs = ain_bf[:, kc:kc + 1] if kc < 8 else uA_bf[:, kc - 8:kc - 7]
                    nc.tensor.matmul(px[:], cwt[:, kc, hc], rhs,
                                     start=(kc == 0), stop=(kc == KC_IH - 1))
                xs = const.tile([P, 1], dt, tag="xs")
                nc.scalar.activation(xs[:], px[:], AF.Relu, bias=cb[:, hc:hc + 1])
                nc.vector.tensor_copy(x_bf[:, hc:hc + 1], xs[:])

            # ================= Stage C: LSTM step =================
            bias2 = const.tile([P, 4], dt)
            nc.vector.tensor_add(bias2[:], bih, bhh)
            gates = []
            for g in range(4):
                pg = ps.tile([P, 1], dt, tag="ps")
                for kc in range(KC_IH):
                    rhs = x_bf[:, kc:kc + 1] if kc < 8 else ain_bf[:, kc:kc + 1]
                    nc.tensor.matmul(pg[:], gwt[:, g, kc], rhs,
                                     start=(kc == 0), stop=(kc == KC_IH - 1))
                act = AF.Tanh if g == 2 else AF.Sigmoid
                gs = const.tile([P, 1], dt, tag=f"gate{g}")
                nc.scalar.activation(gs[:], pg[:], act, bias=bias2[:, g:g + 1])
                gates.append(gs)
            i_g, f_g, g_g, o_g = gates
            t1 = const.tile([P, 1], dt)
            nc.vector.tensor_mul(t1[:], f_g[:], c0)
            t2 = const.tile([P, 1], dt)
            nc.vector.tensor_mul(t2[:], i_g[:], g_g[:])
            c1 = const.tile([P, 1], dt)
            nc.vector.tensor_add(c1[:], t1[:], t2[:])
            tc1 = const.tile([P, 1], dt)
            nc.scalar.activation(tc1[:], c1[:], AF.Tanh)
            h1 = const.tile([P, 1], dt)
            nc.vector.tensor_mul(h1[:], o_g[:], tc1[:])
            nc.gpsimd.dma_start(c_o[:], c1[:])
            nc.gpsimd.dma_start(h_o[:], h1[:])
            h1_bf = const.tile([P, 1], bt)
            nc.vector.tensor_copy(h1_bf[:], h1[:])

            # ==== Stage D: partial logits for the FULL vocab (h-sharded) ====
            logits = const.tile([P, MC_D], dt)
            for g8 in range(N_GRP):
                owt_t = owp.tile([P, D_GRP, P], bt, tag="ow")
                nc.sync.dma_start(
                    owt_t[:], owt_d[:, g8 * D_GRP:(g8 + 1) * D_GRP, :])
                pd = psd.tile([P, D_GRP], dt, tag="pd")
                for j in range(D_GRP):
                    nc.tensor.matmul(pd[:, j:j + 1], owt_t[:, j], h1_bf[:],
                                     start=True, stop=True)
                nc.vector.tensor_copy(
                    logits[:, g8 * D_GRP:(g8 + 1) * D_GRP], pd[:])

            # CC2: AllReduce partial logits (p-major payload, 205KB)
            nc.gpsimd.dma_start(ar_i.rearrange("(p mc) -> p mc", p=P), logits[:])
            nc.gpsimd.collective_compute(
                "AllReduce", mybir.AluOpType.add, replica_groups=RG,
                ins=[ar_i.ap().opt()], outs=[ar_o.ap().opt()])
            lfull = const.tile([P, MC_D], dt)
            nc.gpsimd.dma_start(lfull[:], ar_o.rearrange("(p mc) -> p mc", p=P))
            nc.vector.tensor_add(lfull[:], lfull[:], ob)

            # local log-softmax over the full padded vocab
            exps = const.tile([P, MC_D], dt)
            nc.scalar.activation(exps[:], lfull[:], AF.Exp)
            rowsum = const.tile([P, 1], dt)
            nc.vector.reduce_sum(rowsum[:], exps[:], axis=mybir.AxisListType.X)
            psv = ps.tile([P, 1], dt, tag="ps")
            nc.tensor.matmul(psv[:], ones128[:], rowsum[:])
            sv128 = const.tile([P, 1], dt)
            nc.vector.tensor_copy(sv128[:], psv[:])
            logzb = const.tile([P, 1], dt)
            nc.scalar.activation(logzb[:], sv128[:], AF.Ln)

            # full logp out (p-major); the host slices each core's range
            logp = const.tile([P, MC_D], dt)
            nc.vector.tensor_scalar_sub(logp[:], lfull[:], logzb[:])
            nc.gpsimd.dma_start(logp_o.rearrange("(p mc) -> p mc", p=P), logp[:])

    nc.compile()
    return nc


def _prep_inputs(x):
    """Host-side shard/transpose/tile. Pure data movement + one row gather."""
    tok = int(np.asarray(x["input_token"]).reshape(-1)[0])
    emb_row = np.asarray(x["emb"])[tok].astype(F32)          # [1024]
    h0 = np.asarray(x["h"], F32).reshape(H)
    c0 = np.asarray(x["c"], F32).reshape(H)
    attn_in = np.concatenate([emb_row, h0])                   # [2048]
    attn_W = np.asarray(x["attn_W"], F32)
    attn_b = np.asarray(x["attn_b"], F32)
    enc = np.asarray(x["encoder_outputs"], F32)
    comb_W = np.asarray(x["comb_W"], F32)
    comb_b = np.asarray(x["comb_b"], F32)
    Wcat = np.concatenate([np.asarray(x["W_ih"], F32),
                           np.asarray(x["W_hh"], F32)], axis=1)  # [4096, 2048]
    b_ih = np.asarray(x["b_ih"], F32).reshape(4, NCORES, P)
    b_hh = np.asarray(x["b_hh"], F32).reshape(4, NCORES, P)
    out_W = np.asarray(x["out_W"], F32)
    out_b = np.asarray(x["out_b"], F32)
    ow_pad = np.zeros((V_PAD, H), F32)
    ow_pad[:V] = out_W
    ob_pad = np.full((V_PAD,), -1e30, F32)
    ob_pad[:V] = out_b

    ain_c = np.ascontiguousarray(attn_in.reshape(16, P).T)    # [128,16]
    # comb replicated: [j, kc, hc, p] = comb_W[hc*128+p, kc*128+j]
    cwt = np.ascontiguousarray(
        comb_W.reshape(KC_H, P, KC_IH, P).transpose(3, 2, 0, 1).astype(BF16))

    in_maps = []
    for k in range(NCORES):
        A_k = attn_W[k * LK:(k + 1) * LK]
        awt = np.ascontiguousarray(
            A_k.reshape(MC_A, P, KC_IH, P).transpose(3, 0, 2, 1).astype(BF16))
        enc_aug = np.concatenate(
            [enc[k * LK:(k + 1) * LK], np.ones((LK, P), F32)], axis=1)
        enc_t = np.ascontiguousarray(
            enc_aug.reshape(MC_A, P, H + P).transpose(1, 0, 2).astype(BF16))
        gwt = np.empty((P, 4, KC_IH, P), BF16)
        for g in range(4):
            G = Wcat[g * H + k * P: g * H + (k + 1) * P]
            gwt[:, g] = G.reshape(P, KC_IH, P).transpose(2, 1, 0).astype(BF16)
        # out_W h-sharded: [j, mc, v] = ow_pad[mc*128+v, k*128+j]
        owt = np.ascontiguousarray(
            ow_pad[:, k * P:(k + 1) * P]
            .reshape(MC_D, P, P).transpose(2, 0, 1).astype(BF16))
        small = np.zeros((P, SMALL_COLS), F32)
        small[:, C_AIN:C_AIN + 16] = ain_c
        small[:, C_AB:C_AB + 4] = attn_b[k * LK:(k + 1) * LK].reshape(MC_A, P).T
        small[:, C_CB:C_CB + 8] = comb_b.reshape(KC_H, P).T
        small[:, C_BIH:C_BIH + 4] = b_ih[:, k, :].T
        small[:, C_BHH:C_BHH + 4] = b_hh[:, k, :].T
        small[:, C_C0] = c0[k * P:(k + 1) * P]
        small[:, C_OB:C_OB + MC_D] = ob_pad.reshape(MC_D, P).T
        in_maps.append({
            "awt": awt, "enc": enc_t, "cwt": cwt, "gwt": gwt,
            "owt": owt, "small": np.ascontiguousarray(small),
        })
    return in_maps


def _install_ntff_hook():
    """Recreate the missing antenv.axon_hooks shim so trace=True works."""
    import types, ctypes, contextlib

    if "antenv.axon_hooks" in sys.modules:
        return
    so_path = "/opt/axon/libaxon_pjrt.so"
    lib = ctypes.CDLL(so_path)
    if not hasattr(lib, "axon_start_nrt_profile"):
        return
    lib.axon_start_nrt_profile.argtypes = [ctypes.POINTER(ctypes.c_int64),
                                           ctypes.c_size_t]
    lib.axon_start_nrt_profile.restype = ctypes.c_int64
    lib.axon_stop_nrt_profile.argtypes = [ctypes.c_char_p]
    lib.axon_stop_nrt_profile.restype = ctypes.c_int64

    @contextlib.contextmanager
    def _hook(output_dir, device_ids):
        import jax
        jax.devices()
        if device_ids:
            ids = (ctypes.c_int64 * len(device_ids))(*device_ids)
            rc = lib.axon_start_nrt_profile(ids, len(device_ids))
        else:
            rc = lib.axon_start_nrt_profile(None, 0)
        if rc != 0:
            raise RuntimeError(f"axon_start_nrt_profile rc={rc}")
        try:
            yield
        finally:
            n = lib.axon_stop_nrt_profile(str(output_dir).encode())
            print(f"ntff profile: {n} file(s) written to {output_dir}",
                  file=sys.stderr)

    mod = types.ModuleType("antenv.axon_hooks")
    mod._hook = _hook
    mod.get_axon_ntff_profile_hook = lambda: _hook
    mod.set_axon_ntff_profile_hook = lambda h: None
    sys.modules["antenv.axon_hooks"] = mod


def _run(in_maps, trace=False, **kw):
    from concourse import bass_utils
    if trace:
        _install_ntff_hook()
    if "nc" not in _CACHE:
        _CACHE["nc"] = _build_program()
    return bass_utils.run_bass_kernel_spmd(
        _CACHE["nc"], in_maps, core_ids=list(range(NCORES)), trace=trace, **kw)


def _logp_slice(res_k, k):
    full = res_k["logp_o"].reshape(P, MC_D).T.reshape(-1)   # vocab order
    return full[k * VK:(k + 1) * VK]


def kernel(**inputs):
    in_maps = _prep_inputs(inputs)
    res = _run(in_maps)
    logp = np.concatenate(
        [_logp_slice(res.results[k], k) for k in range(NCORES)])[:V].reshape(1, V)
    attnw = np.concatenate([r["attnw_o"] for r in res.results]).reshape(1, L)
    h1 = np.concatenate([r["h_o"] for r in res.results]).reshape(1, 1, H)
    c1 = np.concatenate([r["c_o"] for r in res.results]).reshape(1, 1, H)
    return logp, (h1, c1), attnw


# revision 24
# speedup vs baseline: 1.2728x; 1.1162x over previous
"""AttnDecoderRNN single-step decoder on 8 Trainium2 NeuronCores.

Tensor-parallel with only TWO collectives on the serial chain:
  A) attn logits slice [512] = attn_in @ attn_W_k.T  (attn_W/enc L-sharded)
     w = exp(logit + b); u_partial = w @ enc_k; S via ones-block column
     -> AllGather [u(1024)|S(x128)] + local sum  (collective #1, 4.6KB)
  B) x = relu(comb_W @ [emb; u/S] + b)  -- comb_W REPLICATED so every core
     has the full x (no collective)
  C) LSTM gate slices (4H g-sharded) -> c1,h1 slices locally (no collective)
  D) out_W sharded along the CONTRACTION (h) dim: core k needs only its own
     h1 slice -> partial logits for the FULL padded vocab [51200]
     -> AllReduce logits (collective #2, 205KB) -> local log-softmax ->
     every core writes full logp; the host slices per-core ranges.
Big weights are bf16 (fp32 PSUM accumulation), host pre-tiled so every DMA
is contiguous per partition. The embedding row is gathered on the host
(4KB of the 206MB table). Vocab padded to 51200 with -1e30 bias (exp->0).
"""

import sys

if "/opt/trn_rl_repo" not in sys.path:
    sys.path.insert(0, "/opt/trn_rl_repo")

import numpy as np
import ml_dtypes

P = 128
I, H, L, V = 1024, 1024, 4096, 50257
NCORES = 8
LK = L // NCORES            # 512 attn rows / core
MC_A = LK // P              # 4
KC_IH = (I + H) // P        # 16 contraction chunks over 2048
KC_H = H // P               # 8
V_PAD = 51200
MC_D = V_PAD // P           # 400 vocab chunks (full padded vocab per core)
VK = V_PAD // NCORES        # 6400 output rows / core
D_GRP = 8                   # vocab chunks per psum/dma group in stage D
N_GRP = MC_D // D_GRP       # 50

F32 = np.float32
BF16 = ml_dtypes.bfloat16

_CACHE = {}

# small-tensor column map
C_AIN, C_AB, C_CB, C_BIH, C_BHH, C_C0, C_OB = 0, 16, 20, 28, 32, 36, 37
SMALL_COLS = C_OB + MC_D    # 437


def _build_program():
    import concourse.bass as bass  # noqa: F401
    import concourse.mybir as mybir
    import concourse.tile as tile
    from concourse import bacc

    dt = mybir.dt.float32
    bt = mybir.dt.bfloat16
    AF = mybir.ActivationFunctionType

    nc = bacc.Bacc("TRN2", target_bir_lowering=False, debug=False,
                   num_devices=NCORES, enable_asserts=False)

    # ---- I/O ---- (all weight layouts are partition(j)-major contiguous)
    awt_d = nc.dram_tensor("awt", [P, MC_A, KC_IH, P], bt, kind="ExternalInput")
    enc_d = nc.dram_tensor("enc", [P, MC_A, H + P], bt, kind="ExternalInput")
    cwt_d = nc.dram_tensor("cwt", [P, KC_IH, KC_H, P], bt, kind="ExternalInput")
    gwt_d = nc.dram_tensor("gwt", [P, 4, KC_IH, P], bt, kind="ExternalInput")
    owt_d = nc.dram_tensor("owt", [P, MC_D, P], bt, kind="ExternalInput")
    small_d = nc.dram_tensor("small", [P, SMALL_COLS], dt, kind="ExternalInput")

    logp_o = nc.dram_tensor("logp_o", [P * MC_D], dt, kind="ExternalOutput")
    attnw_o = nc.dram_tensor("attnw_o", [LK], dt, kind="ExternalOutput")
    h_o = nc.dram_tensor("h_o", [P], dt, kind="ExternalOutput")
    c_o = nc.dram_tensor("c_o", [P], dt, kind="ExternalOutput")

    warm_i = nc.dram_tensor("warm_i", [1], dt)
    warm_o = nc.dram_tensor("warm_o", [1], dt, addr_space="Shared")
    cc1_i = nc.dram_tensor("cc1_i", [H + P], dt)
    cc1_o = nc.dram_tensor("cc1_o", [H + P], dt, addr_space="Shared")
    ar_i = nc.dram_tensor("ar_i", [P * MC_D], bt)
    ar_o = nc.dram_tensor("ar_o", [P * MC_D], bt, addr_space="Shared")
    RG = [list(range(NCORES))]

    from concourse.bass import _add_dep_helper

    with tile.TileContext(nc) as tc:
        with (
            tc.tile_pool(name="const", bufs=1) as const,
            tc.tile_pool(name="ow", bufs=N_GRP) as owp,
            tc.tile_pool(name="ps", bufs=2, space="PSUM") as ps,
            tc.tile_pool(name="psd", bufs=4, space="PSUM") as psd,
        ):
            # ncfw pays ~35us of one-time warmup on the FIRST collective;
            # absorb it under the weight loads with a dummy gather.
            nc.gpsimd.collective_compute(
                "AllReduce", mybir.AluOpType.add,
                replica_groups=[[r] for r in range(NCORES)],
                ins=[warm_i.ap().opt()], outs=[warm_o.ap().opt()])

            # ---- resident weight loads ----
            small = const.tile([P, SMALL_COLS], dt)
            nc.gpsimd.dma_start(small[:], small_d[:])
            awt = const.tile([P, MC_A, KC_IH, P], bt)
            nc.sync.dma_start(awt[:], awt_d[:])
            enc = const.tile([P, MC_A, H + P], bt)
            nc.sync.dma_start(enc[:], enc_d[:])
            cwt = const.tile([P, KC_IH, KC_H, P], bt)
            cwt_dma = nc.sync.dma_start(cwt[:], cwt_d[:])
            gwt = const.tile([P, 4, KC_IH, P], bt)
            gwt_dma = nc.sync.dma_start(gwt[:], gwt_d[:])

            ones128 = const.tile([P, P], dt)
            nc.vector.memset(ones128[:], 1.0)

            ain = small[:, C_AIN:C_AIN + 16]   # [emb(8) | h0(8)] cols
            ab = small[:, C_AB:C_AB + 4]
            cb = small[:, C_CB:C_CB + 8]
            bih = small[:, C_BIH:C_BIH + 4]
            bhh = small[:, C_BHH:C_BHH + 4]
            c0 = small[:, C_C0:C_C0 + 1]
            ob = small[:, C_OB:C_OB + MC_D]
            ain_bf = const.tile([P, 16], bt)
            nc.vector.tensor_copy(ain_bf[:], ain)

            # ================= Stage A: attention =================
            wT = const.tile([P, MC_A], dt)
            wT_bf = const.tile([P, MC_A], bt)
            for mc in range(MC_A):
                pa = ps.tile([P, 1], dt, tag="ps")
                for kc in range(KC_IH):
                    nc.tensor.matmul(pa[:], awt[:, mc, kc], ain_bf[:, kc:kc + 1],
                                     start=(kc == 0), stop=(kc == KC_IH - 1))
                nc.scalar.activation(wT[:, mc:mc + 1], pa[:], AF.Exp,
                                     bias=ab[:, mc:mc + 1])
                nc.vector.tensor_copy(wT_bf[:, mc:mc + 1], wT[:, mc:mc + 1])

            # u[h] = sum_l w[l] enc[l, h]; chunk 8 hits the ones block -> S
            KA = KC_H + 1
            uT = const.tile([P, KA], dt)
            for hc in range(KA):
                pu = ps.tile([P, 1], dt, tag="ps")
                for mc in range(MC_A):
                    nc.tensor.matmul(pu[:], enc[:, mc, hc * P:(hc + 1) * P],
                                     wT_bf[:, mc:mc + 1],
                                     start=(mc == 0), stop=(mc == MC_A - 1))
                nc.vector.tensor_copy(uT[:, hc:hc + 1], pu[:])

            # CC1: AllReduce [u | S] (ncfw entry already paid by warmup)
            cc1w = nc.gpsimd.dma_start(cc1_i.rearrange("(hc p) -> p hc", p=P), uT[:])
            nc.gpsimd.collective_compute(
                "AllReduce", mybir.AluOpType.add, replica_groups=RG,
                ins=[cc1_i.ap().opt()], outs=[cc1_o.ap().opt()])
            uTs = const.tile([P, KA], dt)
            nc.gpsimd.dma_start(uTs[:], cc1_o.rearrange("(hc p) -> p hc", p=P))
            sinv128 = const.tile([P, 1], dt)
            nc.vector.reciprocal(sinv128[:], uTs[:, KC_H:KA])

            wout = const.tile([P, MC_A], dt)
            nc.vector.tensor_scalar_mul(wout[:], wT[:], sinv128[:])
            nc.gpsimd.dma_start(attnw_o.rearrange("(mc p) -> p mc", p=P), wout[:])
            uA = const.tile([P, KC_H], dt)
            nc.vector.tensor_scalar_mul(uA[:], uTs[:, 0:KC_H], sinv128[:])
            uA_bf = const.tile([P, KC_H], bt)
            nc.vector.tensor_copy(uA_bf[:], uA[:])

            # ========== Stage B: combine + relu (replicated: full x) ==========
            x_bf = const.tile([P, KC_H], bt)
            for hc in range(KC_H):
                px = ps.tile([P, 1], dt, tag="ps")
                for kc in range(KC_IH):
                    rhs = ain_bf[:, kc:kc + 1] if kc < 8 else uA_bf[:, kc - 8:kc - 7]
                    nc.tensor.matmul(px[:], cwt[:, kc, hc], rhs,
                                     start=(kc == 0), stop=(kc == KC_IH - 1))
                xs = const.tile([P, 1], dt, tag="xs")
                nc.scalar.activation(xs[:], px[:], AF.Relu, bias=cb[:, hc:hc + 1])
                nc.vector.tensor_copy(x_bf[:, hc:hc + 1], xs[:])

            # ================= Stage C: LSTM step =================
            bias2 = const.tile([P, 4], dt)
            nc.vector.tensor_add(bias2[:], bih, bhh)
            gates = []
            for g in range(4):
                pg = ps.tile([P, 1], dt, tag="ps")
                for kc in range(KC_IH):
                    rhs = x_bf[:, kc:kc + 1] if kc < 8 else ain_bf[:, kc:kc + 1]
                    nc.tensor.matmul(pg[:], gwt[:, g, kc], rhs,
                                     start=(kc == 0), stop=(kc == KC_IH - 1))
                act = AF.Tanh if g == 2 else AF.Sigmoid
                gs = const.tile([P, 1], dt, tag=f"gate{g}")
                nc.scalar.activation(gs[:], pg[:], act, bias=bias2[:, g:g + 1])
                gates.append(gs)
            i_g, f_g, g_g, o_g = gates
            t1 = const.tile([P, 1], dt)
            nc.vector.tensor_mul(t1[:], f_g[:], c0)
            t2 = const.tile([P, 1], dt)
            nc.vector.tensor_mul(t2[:], i_g[:], g_g[:])
            c1 = const.tile([P, 1], dt)
            nc.vector.tensor_add(c1[:], t1[:], t2[:])
            tc1 = const.tile([P, 1], dt)
            nc.scalar.activation(tc1[:], c1[:], AF.Tanh)
            h1 = const.tile([P, 1], dt)
            nc.vector.tensor_mul(h1[:], o_g[:], tc1[:])
            nc.gpsimd.dma_start(c_o[:], c1[:])
            nc.gpsimd.dma_start(h_o[:], h1[:])
            h1_bf = const.tile([P, 1], bt)
            nc.vector.tensor_copy(h1_bf[:], h1[:])

            # ==== Stage D: partial logits for the FULL vocab (h-sharded) ====
            logits = const.tile([P, MC_D], dt)
            ow_dmas = []
            OW_PACE = 6   # completion-chained: caps in-flight depth so the
            for g8 in range(N_GRP):   # collective meshes see spare bandwidth
                owt_t = owp.tile([P, D_GRP, P], bt, tag="ow")
                ow_dma = nc.sync.dma_start(
                    owt_t[:], owt_d[:, g8 * D_GRP:(g8 + 1) * D_GRP, :])
                if g8 >= OW_PACE:
                    _add_dep_helper(ow_dma.ins, ow_dmas[g8 - OW_PACE].ins,
                                    reason="pace ow stream")
                ow_dmas.append(ow_dma)
                pd = psd.tile([P, D_GRP], dt, tag="pd")
                for j in range(D_GRP):
                    nc.tensor.matmul(pd[:, j:j + 1], owt_t[:, j], h1_bf[:],
                                     start=True, stop=True)
                nc.vector.tensor_copy(
                    logits[:, g8 * D_GRP:(g8 + 1) * D_GRP], pd[:])

            # CC2: AllReduce partial logits (p-major payload, 205KB)
            nc.gpsimd.dma_start(ar_i.rearrange("(p mc) -> p mc", p=P), logits[:])
            nc.gpsimd.collective_compute(
                "AllReduce", mybir.AluOpType.add, replica_groups=RG,
                ins=[ar_i.ap().opt()], outs=[ar_o.ap().opt()])
            lfull = const.tile([P, MC_D], dt)
            nc.gpsimd.dma_start(lfull[:], ar_o.rearrange("(p mc) -> p mc", p=P))
            nc.vector.tensor_add(lfull[:], lfull[:], ob)

            # local log-softmax over the full padded vocab
            exps = const.tile([P, MC_D], dt)
            nc.scalar.activation(exps[:], lfull[:], AF.Exp)
            rowsum = const.tile([P, 1], dt)
            nc.vector.reduce_sum(rowsum[:], exps[:], axis=mybir.AxisListType.X)
            psv = ps.tile([P, 1], dt, tag="ps")
            nc.tensor.matmul(psv[:], ones128[:], rowsum[:])
            sv128 = const.tile([P, 1], dt)
            nc.vector.tensor_copy(sv128[:], psv[:])
            logzb = const.tile([P, 1], dt)
            nc.scalar.activation(logzb[:], sv128[:], AF.Ln)

            # full logp out (p-major); the host slices each core's range
            logp = const.tile([P, MC_D], dt)
            nc.vector.tensor_scalar_sub(logp[:], lfull[:], logzb[:])
            nc.gpsimd.dma_start(logp_o.rearrange("(p mc) -> p mc", p=P), logp[:])

    nc.compile()
    return nc


def _prep_inputs(x):
    """Host-side shard/transpose/tile. Pure data movement + one row gather."""
    tok = int(np.asarray(x["input_token"]).reshape(-1)[0])
    emb_row = np.asarray(x["emb"])[tok].astype(F32)          # [1024]
    h0 = np.asarray(x["h"], F32).reshape(H)
    c0 = np.asarray(x["c"], F32).reshape(H)
    attn_in = np.concatenate([emb_row, h0])                   # [2048]
    attn_W = np.asarray(x["attn_W"], F32)
    attn_b = np.asarray(x["attn_b"], F32)
    enc = np.asarray(x["encoder_outputs"], F32)
    comb_W = np.asarray(x["comb_W"], F32)
    comb_b = np.asarray(x["comb_b"], F32)
    Wcat = np.concatenate([np.asarray(x["W_ih"], F32),
                           np.asarray(x["W_hh"], F32)], axis=1)  # [4096, 2048]
    b_ih = np.asarray(x["b_ih"], F32).reshape(4, NCORES, P)
    b_hh = np.asarray(x["b_hh"], F32).reshape(4, NCORES, P)
    out_W = np.asarray(x["out_W"], F32)
    out_b = np.asarray(x["out_b"], F32)
    ow_pad = np.zeros((V_PAD, H), F32)
    ow_pad[:V] = out_W
    ob_pad = np.full((V_PAD,), -1e30, F32)
    ob_pad[:V] = out_b

    ain_c = np.ascontiguousarray(attn_in.reshape(16, P).T)    # [128,16]
    # comb replicated: [j, kc, hc, p] = comb_W[hc*128+p, kc*128+j]
    cwt = np.ascontiguousarray(
        comb_W.reshape(KC_H, P, KC_IH, P).transpose(3, 2, 0, 1).astype(BF16))

    in_maps = []
    for k in range(NCORES):
        A_k = attn_W[k * LK:(k + 1) * LK]
        awt = np.ascontiguousarray(
            A_k.reshape(MC_A, P, KC_IH, P).transpose(3, 0, 2, 1).astype(BF16))
        enc_aug = np.concatenate(
            [enc[k * LK:(k + 1) * LK], np.ones((LK, P), F32)], axis=1)
        enc_t = np.ascontiguousarray(
            enc_aug.reshape(MC_A, P, H + P).transpose(1, 0, 2).astype(BF16))
        gwt = np.empty((P, 4, KC_IH, P), BF16)
        for g in range(4):
            G = Wcat[g * H + k * P: g * H + (k + 1) * P]
            gwt[:, g] = G.reshape(P, KC_IH, P).transpose(2, 1, 0).astype(BF16)
        # out_W h-sharded: [j, mc, v] = ow_pad[mc*128+v, k*128+j]
        owt = np.ascontiguousarray(
            ow_pad[:, k * P:(k + 1) * P]
            .reshape(MC_D, P, P).transpose(2, 0, 1).astype(BF16))
        small = np.zeros((P, SMALL_COLS), F32)
        small[:, C_AIN:C_AIN + 16] = ain_c
        small[:, C_AB:C_AB + 4] = attn_b[k * LK:(k + 1) * LK].reshape(MC_A, P).T
        small[:, C_CB:C_CB + 8] = comb_b.reshape(KC_H, P).T
        small[:, C_BIH:C_BIH + 4] = b_ih[:, k, :].T
        small[:, C_BHH:C_BHH + 4] = b_hh[:, k, :].T
        small[:, C_C0] = c0[k * P:(k + 1) * P]
        small[:, C_OB:C_OB + MC_D] = ob_pad.reshape(MC_D, P).T
        in_maps.append({
            "awt": awt, "enc": enc_t, "cwt": cwt, "gwt": gwt,
            "owt": owt, "small": np.ascontiguousarray(small),
        })
    return in_maps


def _install_ntff_hook():
    """Recreate the missing antenv.axon_hooks shim so trace=True works."""
    import types, ctypes, contextlib

    if "antenv.axon_hooks" in sys.modules:
        return
    so_path = "/opt/axon/libaxon_pjrt.so"
    lib = ctypes.CDLL(so_path)
    if not hasattr(lib, "axon_start_nrt_profile"):
        return
    lib.axon_start_nrt_profile.argtypes = [ctypes.POINTER(ctypes.c_int64),
                                           ctypes.c_size_t]
    lib.axon_start_nrt_profile.restype = ctypes.c_int64
    lib.axon_stop_nrt_profile.argtypes = [ctypes.c_char_p]
    lib.axon_stop_nrt_profile.restype = ctypes.c_int64

    @contextlib.contextmanager
    def _hook(output_dir, device_ids):
        import jax
        jax.devices()
        if device_ids:
            ids = (ctypes.c_int64 * len(device_ids))(*device_ids)
            rc = lib.axon_start_nrt_profile(ids, len(device_ids))
        else:
            rc = lib.axon_start_nrt_profile(None, 0)
        if rc != 0:
            raise RuntimeError(f"axon_start_nrt_profile rc={rc}")
        try:
            yield
        finally:
            n = lib.axon_stop_nrt_profile(str(output_dir).encode())
            print(f"ntff profile: {n} file(s) written to {output_dir}",
                  file=sys.stderr)

    mod = types.ModuleType("antenv.axon_hooks")
    mod._hook = _hook
    mod.get_axon_ntff_profile_hook = lambda: _hook
    mod.set_axon_ntff_profile_hook = lambda h: None
    sys.modules["antenv.axon_hooks"] = mod


def _run(in_maps, trace=False, **kw):
    from concourse import bass_utils
    if trace:
        _install_ntff_hook()
    if "nc" not in _CACHE:
        _CACHE["nc"] = _build_program()
    return bass_utils.run_bass_kernel_spmd(
        _CACHE["nc"], in_maps, core_ids=list(range(NCORES)), trace=trace, **kw)


def _logp_slice(res_k, k):
    full = res_k["logp_o"].reshape(P, MC_D).T.reshape(-1)   # vocab order
    return full[k * VK:(k + 1) * VK]


def kernel(**inputs):
    in_maps = _prep_inputs(inputs)
    res = _run(in_maps)
    logp = np.concatenate(
        [_logp_slice(res.results[k], k) for k in range(NCORES)])[:V].reshape(1, V)
    attnw = np.concatenate([r["attnw_o"] for r in res.results]).reshape(1, L)
    h1 = np.concatenate([r["h_o"] for r in res.results]).reshape(1, 1, H)
    c1 = np.concatenate([r["c_o"] for r in res.results]).reshape(1, 1, H)
    return logp, (h1, c1), attnw


# revision 25
# speedup vs baseline: 1.2839x; 1.0087x over previous
"""AttnDecoderRNN single-step decoder on 8 Trainium2 NeuronCores.

Tensor-parallel with only TWO collectives on the serial chain:
  A) attn logits slice [512] = attn_in @ attn_W_k.T  (attn_W/enc L-sharded)
     w = exp(logit + b); u_partial = w @ enc_k; S via ones-block column
     -> AllGather [u(1024)|S(x128)] + local sum  (collective #1, 4.6KB)
  B) x = relu(comb_W @ [emb; u/S] + b)  -- comb_W REPLICATED so every core
     has the full x (no collective)
  C) LSTM gate slices (4H g-sharded) -> c1,h1 slices locally (no collective)
  D) out_W sharded along the CONTRACTION (h) dim: core k needs only its own
     h1 slice -> partial logits for the FULL padded vocab [51200]
     -> AllReduce logits (collective #2, 205KB) -> local log-softmax ->
     every core writes full logp; the host slices per-core ranges.
Big weights are bf16 (fp32 PSUM accumulation), host pre-tiled so every DMA
is contiguous per partition. The embedding row is gathered on the host
(4KB of the 206MB table). Vocab padded to 51200 with -1e30 bias (exp->0).
"""

import sys

if "/opt/trn_rl_repo" not in sys.path:
    sys.path.insert(0, "/opt/trn_rl_repo")

import numpy as np
import ml_dtypes

P = 128
I, H, L, V = 1024, 1024, 4096, 50257
NCORES = 8
LK = L // NCORES            # 512 attn rows / core
MC_A = LK // P              # 4
KC_IH = (I + H) // P        # 16 contraction chunks over 2048
KC_H = H // P               # 8
V_PAD = 51200
MC_D = V_PAD // P           # 400 vocab chunks (full padded vocab per core)
VK = V_PAD // NCORES        # 6400 output rows / core
D_GRP = 8                   # vocab chunks per psum/dma group in stage D
N_GRP = MC_D // D_GRP       # 50

F32 = np.float32
BF16 = ml_dtypes.bfloat16

_CACHE = {}

# small-tensor column map
C_AIN, C_AB, C_CB, C_BIH, C_BHH, C_C0, C_OB = 0, 16, 20, 28, 32, 36, 37
SMALL_COLS = C_OB + MC_D    # 437


def _build_program():
    import concourse.bass as bass  # noqa: F401
    import concourse.mybir as mybir
    import concourse.tile as tile
    from concourse import bacc

    dt = mybir.dt.float32
    bt = mybir.dt.bfloat16
    AF = mybir.ActivationFunctionType

    nc = bacc.Bacc("TRN2", target_bir_lowering=False, debug=False,
                   num_devices=NCORES, enable_asserts=False)

    # ---- I/O ---- (all weight layouts are partition(j)-major contiguous)
    awt_d = nc.dram_tensor("awt", [P, MC_A, KC_IH, P], bt, kind="ExternalInput")
    enc_d = nc.dram_tensor("enc", [P, MC_A, H + P], bt, kind="ExternalInput")
    cwt_d = nc.dram_tensor("cwt", [P, KC_IH, KC_H, P], bt, kind="ExternalInput")
    gwt_d = nc.dram_tensor("gwt", [P, 4, KC_IH, P], bt, kind="ExternalInput")
    owt_d = nc.dram_tensor("owt", [P, MC_D, P], bt, kind="ExternalInput")
    small_d = nc.dram_tensor("small", [P, SMALL_COLS], dt, kind="ExternalInput")

    logp_o = nc.dram_tensor("logp_o", [P * MC_D], dt, kind="ExternalOutput")
    attnw_o = nc.dram_tensor("attnw_o", [LK], dt, kind="ExternalOutput")
    h_o = nc.dram_tensor("h_o", [P], dt, kind="ExternalOutput")
    c_o = nc.dram_tensor("c_o", [P], dt, kind="ExternalOutput")

    warm_i = nc.dram_tensor("warm_i", [1], dt)
    warm_o = nc.dram_tensor("warm_o", [1], dt, addr_space="Shared")
    cc1_i = nc.dram_tensor("cc1_i", [H + P], dt)
    cc1_o = nc.dram_tensor("cc1_o", [H + P], dt, addr_space="Shared")
    ar_i = nc.dram_tensor("ar_i", [P * MC_D], bt)
    ar_o = nc.dram_tensor("ar_o", [P * MC_D], bt, addr_space="Shared")
    RG = [list(range(NCORES))]

    from concourse.bass import _add_dep_helper

    with tile.TileContext(nc) as tc:
        with (
            tc.tile_pool(name="const", bufs=1) as const,
            tc.tile_pool(name="ow", bufs=N_GRP) as owp,
            tc.tile_pool(name="ps", bufs=2, space="PSUM") as ps,
            tc.tile_pool(name="psd", bufs=4, space="PSUM") as psd,
        ):
            # ---- resident weight loads ----
            small = const.tile([P, SMALL_COLS], dt)
            nc.gpsimd.dma_start(small[:], small_d[:])
            awt = const.tile([P, MC_A, KC_IH, P], bt)
            nc.sync.dma_start(awt[:], awt_d[:])
            enc = const.tile([P, MC_A, H + P], bt)
            nc.sync.dma_start(enc[:], enc_d[:])
            cwt = const.tile([P, KC_IH, KC_H, P], bt)
            cwt_dma = nc.sync.dma_start(cwt[:], cwt_d[:])
            gwt = const.tile([P, 4, KC_IH, P], bt)
            gwt_dma = nc.sync.dma_start(gwt[:], gwt_d[:])

            ones128 = const.tile([P, P], dt)
            nc.vector.memset(ones128[:], 1.0)

            ain = small[:, C_AIN:C_AIN + 16]   # [emb(8) | h0(8)] cols
            ab = small[:, C_AB:C_AB + 4]
            cb = small[:, C_CB:C_CB + 8]
            bih = small[:, C_BIH:C_BIH + 4]
            bhh = small[:, C_BHH:C_BHH + 4]
            c0 = small[:, C_C0:C_C0 + 1]
            ob = small[:, C_OB:C_OB + MC_D]
            ain_bf = const.tile([P, 16], bt)
            nc.vector.tensor_copy(ain_bf[:], ain)

            # ================= Stage A: attention =================
            wT = const.tile([P, MC_A], dt)
            wT_bf = const.tile([P, MC_A], bt)
            for mc in range(MC_A):
                pa = ps.tile([P, 1], dt, tag="ps")
                for kc in range(KC_IH):
                    nc.tensor.matmul(pa[:], awt[:, mc, kc], ain_bf[:, kc:kc + 1],
                                     start=(kc == 0), stop=(kc == KC_IH - 1))
                nc.scalar.activation(wT[:, mc:mc + 1], pa[:], AF.Exp,
                                     bias=ab[:, mc:mc + 1])
                nc.vector.tensor_copy(wT_bf[:, mc:mc + 1], wT[:, mc:mc + 1])

            # u[h] = sum_l w[l] enc[l, h]; chunk 8 hits the ones block -> S
            KA = KC_H + 1
            uT = const.tile([P, KA], dt)
            for hc in range(KA):
                pu = ps.tile([P, 1], dt, tag="ps")
                for mc in range(MC_A):
                    nc.tensor.matmul(pu[:], enc[:, mc, hc * P:(hc + 1) * P],
                                     wT_bf[:, mc:mc + 1],
                                     start=(mc == 0), stop=(mc == MC_A - 1))
                nc.vector.tensor_copy(uT[:, hc:hc + 1], pu[:])

            # CC1: AllReduce [u | S] (ncfw entry already paid by warmup)
            cc1w = nc.gpsimd.dma_start(cc1_i.rearrange("(hc p) -> p hc", p=P), uT[:])
            nc.gpsimd.collective_compute(
                "AllReduce", mybir.AluOpType.add, replica_groups=RG,
                ins=[cc1_i.ap().opt()], outs=[cc1_o.ap().opt()])
            uTs = const.tile([P, KA], dt)
            nc.gpsimd.dma_start(uTs[:], cc1_o.rearrange("(hc p) -> p hc", p=P))
            sinv128 = const.tile([P, 1], dt)
            nc.vector.reciprocal(sinv128[:], uTs[:, KC_H:KA])

            wout = const.tile([P, MC_A], dt)
            nc.vector.tensor_scalar_mul(wout[:], wT[:], sinv128[:])
            nc.gpsimd.dma_start(attnw_o.rearrange("(mc p) -> p mc", p=P), wout[:])
            uA = const.tile([P, KC_H], dt)
            nc.vector.tensor_scalar_mul(uA[:], uTs[:, 0:KC_H], sinv128[:])
            uA_bf = const.tile([P, KC_H], bt)
            nc.vector.tensor_copy(uA_bf[:], uA[:])

            # ========== Stage B: combine + relu (replicated: full x) ==========
            x_bf = const.tile([P, KC_H], bt)
            for hc in range(KC_H):
                px = ps.tile([P, 1], dt, tag="ps")
                for kc in range(KC_IH):
                    rhs = ain_bf[:, kc:kc + 1] if kc < 8 else uA_bf[:, kc - 8:kc - 7]
                    nc.tensor.matmul(px[:], cwt[:, kc, hc], rhs,
                                     start=(kc == 0), stop=(kc == KC_IH - 1))
                xs = const.tile([P, 1], dt, tag="xs")
                nc.scalar.activation(xs[:], px[:], AF.Relu, bias=cb[:, hc:hc + 1])
                nc.vector.tensor_copy(x_bf[:, hc:hc + 1], xs[:])

            # ================= Stage C: LSTM step =================
            bias2 = const.tile([P, 4], dt)
            nc.vector.tensor_add(bias2[:], bih, bhh)
            gates = []
            for g in range(4):
                pg = ps.tile([P, 1], dt, tag="ps")
                for kc in range(KC_IH):
                    rhs = x_bf[:, kc:kc + 1] if kc < 8 else ain_bf[:, kc:kc + 1]
                    nc.tensor.matmul(pg[:], gwt[:, g, kc], rhs,
                                     start=(kc == 0), stop=(kc == KC_IH - 1))
                act = AF.Tanh if g == 2 else AF.Sigmoid
                gs = const.tile([P, 1], dt, tag=f"gate{g}")
                nc.scalar.activation(gs[:], pg[:], act, bias=bias2[:, g:g + 1])
                gates.append(gs)
            i_g, f_g, g_g, o_g = gates
            t1 = const.tile([P, 1], dt)
            nc.vector.tensor_mul(t1[:], f_g[:], c0)
            t2 = const.tile([P, 1], dt)
            nc.vector.tensor_mul(t2[:], i_g[:], g_g[:])
            c1 = const.tile([P, 1], dt)
            nc.vector.tensor_add(c1[:], t1[:], t2[:])
            tc1 = const.tile([P, 1], dt)
            nc.scalar.activation(tc1[:], c1[:], AF.Tanh)
            h1 = const.tile([P, 1], dt)
            nc.vector.tensor_mul(h1[:], o_g[:], tc1[:])
            nc.gpsimd.dma_start(c_o[:], c1[:])
            nc.gpsimd.dma_start(h_o[:], h1[:])
            h1_bf = const.tile([P, 1], bt)
            nc.vector.tensor_copy(h1_bf[:], h1[:])

            # ==== Stage D: partial logits for the FULL vocab (h-sharded) ====
            logits = const.tile([P, MC_D], dt)
            ow_dmas = []
            OW_PACE = 6   # completion-chained: caps in-flight depth so the
            for g8 in range(N_GRP):   # collective meshes see spare bandwidth
                owt_t = owp.tile([P, D_GRP, P], bt, tag="ow")
                ow_dma = nc.sync.dma_start(
                    owt_t[:], owt_d[:, g8 * D_GRP:(g8 + 1) * D_GRP, :])
                if g8 >= OW_PACE:
                    _add_dep_helper(ow_dma.ins, ow_dmas[g8 - OW_PACE].ins,
                                    reason="pace ow stream")
                ow_dmas.append(ow_dma)
                pd = psd.tile([P, D_GRP], dt, tag="pd")
                for j in range(D_GRP):
                    nc.tensor.matmul(pd[:, j:j + 1], owt_t[:, j], h1_bf[:],
                                     start=True, stop=True)
                nc.vector.tensor_copy(
                    logits[:, g8 * D_GRP:(g8 + 1) * D_GRP], pd[:])

            # CC2: AllReduce partial logits (p-major payload, 205KB)
            nc.gpsimd.dma_start(ar_i.rearrange("(p mc) -> p mc", p=P), logits[:])
            nc.gpsimd.collective_compute(
                "AllReduce", mybir.AluOpType.add, replica_groups=RG,
                ins=[ar_i.ap().opt()], outs=[ar_o.ap().opt()])
            lfull = const.tile([P, MC_D], dt)
            nc.gpsimd.dma_start(lfull[:], ar_o.rearrange("(p mc) -> p mc", p=P))
            nc.vector.tensor_add(lfull[:], lfull[:], ob)

            # local log-softmax over the full padded vocab
            exps = const.tile([P, MC_D], dt)
            nc.scalar.activation(exps[:], lfull[:], AF.Exp)
            rowsum = const.tile([P, 1], dt)
            nc.vector.reduce_sum(rowsum[:], exps[:], axis=mybir.AxisListType.X)
            psv = ps.tile([P, 1], dt, tag="ps")
            nc.tensor.matmul(psv[:], ones128[:], rowsum[:])
            sv128 = const.tile([P, 1], dt)
            nc.vector.tensor_copy(sv128[:], psv[:])
            logzb = const.tile([P, 1], dt)
            nc.scalar.activation(logzb[:], sv128[:], AF.Ln)

            # full logp out (p-major); the host slices each core's range
            logp = const.tile([P, MC_D], dt)
            nc.vector.tensor_scalar_sub(logp[:], lfull[:], logzb[:])
            nc.gpsimd.dma_start(logp_o.rearrange("(p mc) -> p mc", p=P), logp[:])

    nc.compile()
    return nc


def _prep_inputs(x):
    """Host-side shard/transpose/tile. Pure data movement + one row gather."""
    tok = int(np.asarray(x["input_token"]).reshape(-1)[0])
    emb_row = np.asarray(x["emb"])[tok].astype(F32)          # [1024]
    h0 = np.asarray(x["h"], F32).reshape(H)
    c0 = np.asarray(x["c"], F32).reshape(H)
    attn_in = np.concatenate([emb_row, h0])                   # [2048]
    attn_W = np.asarray(x["attn_W"], F32)
    attn_b = np.asarray(x["attn_b"], F32)
    enc = np.asarray(x["encoder_outputs"], F32)
    comb_W = np.asarray(x["comb_W"], F32)
    comb_b = np.asarray(x["comb_b"], F32)
    Wcat = np.concatenate([np.asarray(x["W_ih"], F32),
                           np.asarray(x["W_hh"], F32)], axis=1)  # [4096, 2048]
    b_ih = np.asarray(x["b_ih"], F32).reshape(4, NCORES, P)
    b_hh = np.asarray(x["b_hh"], F32).reshape(4, NCORES, P)
    out_W = np.asarray(x["out_W"], F32)
    out_b = np.asarray(x["out_b"], F32)
    ow_pad = np.zeros((V_PAD, H), F32)
    ow_pad[:V] = out_W
    ob_pad = np.full((V_PAD,), -1e30, F32)
    ob_pad[:V] = out_b

    ain_c = np.ascontiguousarray(attn_in.reshape(16, P).T)    # [128,16]
    # comb replicated: [j, kc, hc, p] = comb_W[hc*128+p, kc*128+j]
    cwt = np.ascontiguousarray(
        comb_W.reshape(KC_H, P, KC_IH, P).transpose(3, 2, 0, 1).astype(BF16))

    in_maps = []
    for k in range(NCORES):
        A_k = attn_W[k * LK:(k + 1) * LK]
        awt = np.ascontiguousarray(
            A_k.reshape(MC_A, P, KC_IH, P).transpose(3, 0, 2, 1).astype(BF16))
        enc_aug = np.concatenate(
            [enc[k * LK:(k + 1) * LK], np.ones((LK, P), F32)], axis=1)
        enc_t = np.ascontiguousarray(
            enc_aug.reshape(MC_A, P, H + P).transpose(1, 0, 2).astype(BF16))
        gwt = np.empty((P, 4, KC_IH, P), BF16)
        for g in range(4):
            G = Wcat[g * H + k * P: g * H + (k + 1) * P]
            gwt[:, g] = G.reshape(P, KC_IH, P).transpose(2, 1, 0).astype(BF16)
        # out_W h-sharded: [j, mc, v] = ow_pad[mc*128+v, k*128+j]
        owt = np.ascontiguousarray(
            ow_pad[:, k * P:(k + 1) * P]
            .reshape(MC_D, P, P).transpose(2, 0, 1).astype(BF16))
        small = np.zeros((P, SMALL_COLS), F32)
        small[:, C_AIN:C_AIN + 16] = ain_c
        small[:, C_AB:C_AB + 4] = attn_b[k * LK:(k + 1) * LK].reshape(MC_A, P).T
        small[:, C_CB:C_CB + 8] = comb_b.reshape(KC_H, P).T
        small[:, C_BIH:C_BIH + 4] = b_ih[:, k, :].T
        small[:, C_BHH:C_BHH + 4] = b_hh[:, k, :].T
        small[:, C_C0] = c0[k * P:(k + 1) * P]
        small[:, C_OB:C_OB + MC_D] = ob_pad.reshape(MC_D, P).T
        in_maps.append({
            "awt": awt, "enc": enc_t, "cwt": cwt, "gwt": gwt,
            "owt": owt, "small": np.ascontiguousarray(small),
        })
    return in_maps


def _install_ntff_hook():
    """Recreate the missing antenv.axon_hooks shim so trace=True works."""
    import types, ctypes, contextlib

    if "antenv.axon_hooks" in sys.modules:
        return
    so_path = "/opt/axon/libaxon_pjrt.so"
    lib = ctypes.CDLL(so_path)
    if not hasattr(lib, "axon_start_nrt_profile"):
        return
    lib.axon_start_nrt_profile.argtypes = [ctypes.POINTER(ctypes.c_int64),
                                           ctypes.c_size_t]
    lib.axon_start_nrt_profile.restype = ctypes.c_int64
    lib.axon_stop_nrt_profile.argtypes = [ctypes.c_char_p]
    lib.axon_stop_nrt_profile.restype = ctypes.c_int64

    @contextlib.contextmanager
    def _hook(output_dir, device_ids):
        import jax
        jax.devices()
        if device_ids:
            ids = (ctypes.c_int64 * len(device_ids))(*device_ids)
            rc = lib.axon_start_nrt_profile(ids, len(device_ids))
        else:
            rc = lib.axon_start_nrt_profile(None, 0)
        if rc != 0:
            raise RuntimeError(f"axon_start_nrt_profile rc={rc}")
        try:
            yield
        finally:
            n = lib.axon_stop_nrt_profile(str(output_dir).encode())
            print(f"ntff profile: {n} file(s) written to {output_dir}",
                  file=sys.stderr)

    mod = types.ModuleType("antenv.axon_hooks")
    mod._hook = _hook
    mod.get_axon_ntff_profile_hook = lambda: _hook
    mod.set_axon_ntff_profile_hook = lambda h: None
    sys.modules["antenv.axon_hooks"] = mod


def _run(in_maps, trace=False, **kw):
    from concourse import bass_utils
    if trace:
        _install_ntff_hook()
    if "nc" not in _CACHE:
        _CACHE["nc"] = _build_program()
    return bass_utils.run_bass_kernel_spmd(
        _CACHE["nc"], in_maps, core_ids=list(range(NCORES)), trace=trace, **kw)


def _logp_slice(res_k, k):
    full = res_k["logp_o"].reshape(P, MC_D).T.reshape(-1)   # vocab order
    return full[k * VK:(k + 1) * VK]


def kernel(**inputs):
    in_maps = _prep_inputs(inputs)
    res = _run(in_maps)
    logp = np.concatenate(
        [_logp_slice(res.results[k], k) for k in range(NCORES)])[:V].reshape(1, V)
    attnw = np.concatenate([r["attnw_o"] for r in res.results]).reshape(1, L)
    h1 = np.concatenate([r["h_o"] for r in res.results]).reshape(1, 1, H)
    c1 = np.concatenate([r["c_o"] for r in res.results]).reshape(1, 1, H)
    return logp, (h1, c1), attnw


# revision 26
# speedup vs baseline: 1.3296x; 1.0356x over previous
"""AttnDecoderRNN single-step decoder on 8 Trainium2 NeuronCores.

Tensor-parallel with only TWO collectives on the serial chain:
  A) attn logits slice [512] = attn_in @ attn_W_k.T  (attn_W/enc L-sharded)
     w = exp(logit + b); u_partial = w @ enc_k; S via ones-block column
     -> AllGather [u(1024)|S(x128)] + local sum  (collective #1, 4.6KB)
  B) x = relu(comb_W @ [emb; u/S] + b)  -- comb_W REPLICATED so every core
     has the full x (no collective)
  C) LSTM gate slices (4H g-sharded) -> c1,h1 slices locally (no collective)
  D) out_W sharded along the CONTRACTION (h) dim: core k needs only its own
     h1 slice -> partial logits for the FULL padded vocab [51200]
     -> AllReduce logits (collective #2, 205KB) -> local log-softmax ->
     every core writes full logp; the host slices per-core ranges.
Big weights are bf16 (fp32 PSUM accumulation), host pre-tiled so every DMA
is contiguous per partition. The embedding row is gathered on the host
(4KB of the 206MB table). Vocab padded to 51200 with -1e30 bias (exp->0).
"""

import sys

if "/opt/trn_rl_repo" not in sys.path:
    sys.path.insert(0, "/opt/trn_rl_repo")

import numpy as np
import ml_dtypes

P = 128
I, H, L, V = 1024, 1024, 4096, 50257
NCORES = 8
LK = L // NCORES            # 512 attn rows / core
MC_A = LK // P              # 4
KC_IH = (I + H) // P        # 16 contraction chunks over 2048
KC_H = H // P               # 8
V_PAD = 51200
MC_D = V_PAD // P           # 400 vocab chunks (full padded vocab per core)
VK = V_PAD // NCORES        # 6400 output rows / core
D_GRP = 8                   # vocab chunks per psum/dma group in stage D
N_GRP = MC_D // D_GRP       # 50

F32 = np.float32
BF16 = ml_dtypes.bfloat16

_CACHE = {}

# small-tensor column map
C_AIN, C_AB, C_CB, C_BIH, C_BHH, C_C0, C_OB = 0, 16, 20, 28, 32, 36, 37
SMALL_COLS = C_OB + MC_D    # 437


def _build_program():
    import concourse.bass as bass  # noqa: F401
    import concourse.mybir as mybir
    import concourse.tile as tile
    from concourse import bacc

    dt = mybir.dt.float32
    bt = mybir.dt.bfloat16
    AF = mybir.ActivationFunctionType

    nc = bacc.Bacc("TRN2", target_bir_lowering=False, debug=False,
                   num_devices=NCORES, enable_asserts=False)

    # ---- I/O ---- (all weight layouts are partition(j)-major contiguous)
    awt_d = nc.dram_tensor("awt", [P, MC_A, KC_IH, P], bt, kind="ExternalInput")
    enc_d = nc.dram_tensor("enc", [P, MC_A, H + P], bt, kind="ExternalInput")
    cwt_d = nc.dram_tensor("cwt", [P, KC_IH, KC_H, P], bt, kind="ExternalInput")
    gwt_d = nc.dram_tensor("gwt", [P, 4, KC_IH, P], bt, kind="ExternalInput")
    owt_d = nc.dram_tensor("owt", [P, MC_D, P], bt, kind="ExternalInput")
    small_d = nc.dram_tensor("small", [P, SMALL_COLS], dt, kind="ExternalInput")

    logp_o = nc.dram_tensor("logp_o", [P * MC_D], dt, kind="ExternalOutput")
    attnw_o = nc.dram_tensor("attnw_o", [LK], dt, kind="ExternalOutput")
    h_o = nc.dram_tensor("h_o", [P], dt, kind="ExternalOutput")
    c_o = nc.dram_tensor("c_o", [P], dt, kind="ExternalOutput")

    warm_i = nc.dram_tensor("warm_i", [1], dt)
    warm_o = nc.dram_tensor("warm_o", [1], dt, addr_space="Shared")
    cc1_i = nc.dram_tensor("cc1_i", [H + P], dt)
    cc1_o = nc.dram_tensor("cc1_o", [H + P], dt, addr_space="Shared")
    ar_i = nc.dram_tensor("ar_i", [P * MC_D], bt)
    ar_o = nc.dram_tensor("ar_o", [P * MC_D], bt, addr_space="Shared")
    RG = [list(range(NCORES))]

    from concourse.bass import _add_dep_helper

    with tile.TileContext(nc) as tc:
        with (
            tc.tile_pool(name="const", bufs=1) as const,
            tc.tile_pool(name="ow", bufs=N_GRP) as owp,
            tc.tile_pool(name="ps", bufs=2, space="PSUM") as ps,
            tc.tile_pool(name="psd", bufs=4, space="PSUM") as psd,
        ):
            # ---- resident weight loads ----
            small = const.tile([P, SMALL_COLS], dt)
            nc.gpsimd.dma_start(small[:], small_d[:])
            awt = const.tile([P, MC_A, KC_IH, P], bt)
            nc.sync.dma_start(awt[:], awt_d[:])
            enc = const.tile([P, MC_A, H + P], bt)
            nc.sync.dma_start(enc[:], enc_d[:])
            cwt = const.tile([P, KC_IH, KC_H, P], bt)
            cwt_dma = nc.sync.dma_start(cwt[:], cwt_d[:])
            gwt = const.tile([P, 4, KC_IH, P], bt)
            gwt_dma = nc.sync.dma_start(gwt[:], gwt_d[:])

            ones128 = const.tile([P, P], dt)
            nc.vector.memset(ones128[:], 1.0)

            ain = small[:, C_AIN:C_AIN + 16]   # [emb(8) | h0(8)] cols
            ab = small[:, C_AB:C_AB + 4]
            cb = small[:, C_CB:C_CB + 8]
            bih = small[:, C_BIH:C_BIH + 4]
            bhh = small[:, C_BHH:C_BHH + 4]
            c0 = small[:, C_C0:C_C0 + 1]
            ob = small[:, C_OB:C_OB + MC_D]
            ain_bf = const.tile([P, 16], bt)
            nc.vector.tensor_copy(ain_bf[:], ain)

            # ================= Stage A: attention =================
            wT = const.tile([P, MC_A], dt)
            wT_bf = const.tile([P, MC_A], bt)
            for mc in range(MC_A):
                pa = ps.tile([P, 1], dt, tag="ps")
                for kc in range(KC_IH):
                    nc.tensor.matmul(pa[:], awt[:, mc, kc], ain_bf[:, kc:kc + 1],
                                     start=(kc == 0), stop=(kc == KC_IH - 1))
                nc.scalar.activation(wT[:, mc:mc + 1], pa[:], AF.Exp,
                                     bias=ab[:, mc:mc + 1])
                nc.vector.tensor_copy(wT_bf[:, mc:mc + 1], wT[:, mc:mc + 1])

            # u[h] = sum_l w[l] enc[l, h]; chunk 8 hits the ones block -> S
            KA = KC_H + 1
            uT = const.tile([P, KA], dt)
            for hc in range(KA):
                pu = ps.tile([P, 1], dt, tag="ps")
                for mc in range(MC_A):
                    nc.tensor.matmul(pu[:], enc[:, mc, hc * P:(hc + 1) * P],
                                     wT_bf[:, mc:mc + 1],
                                     start=(mc == 0), stop=(mc == MC_A - 1))
                nc.vector.tensor_copy(uT[:, hc:hc + 1], pu[:])

            # CC1: AllReduce [u | S] (ncfw entry already paid by warmup)
            cc1w = nc.gpsimd.dma_start(cc1_i.rearrange("(hc p) -> p hc", p=P), uT[:])
            nc.gpsimd.collective_compute(
                "AllReduce", mybir.AluOpType.add, replica_groups=RG,
                ins=[cc1_i.ap().opt()], outs=[cc1_o.ap().opt()])
            uTs = const.tile([P, KA], dt)
            nc.gpsimd.dma_start(uTs[:], cc1_o.rearrange("(hc p) -> p hc", p=P))
            sinv128 = const.tile([P, 1], dt)
            nc.vector.reciprocal(sinv128[:], uTs[:, KC_H:KA])

            wout = const.tile([P, MC_A], dt)
            nc.vector.tensor_scalar_mul(wout[:], wT[:], sinv128[:])
            nc.gpsimd.dma_start(attnw_o.rearrange("(mc p) -> p mc", p=P), wout[:])
            uA = const.tile([P, KC_H], dt)
            nc.vector.tensor_scalar_mul(uA[:], uTs[:, 0:KC_H], sinv128[:])
            uA_bf = const.tile([P, KC_H], bt)
            nc.vector.tensor_copy(uA_bf[:], uA[:])

            # ========== Stage B: combine + relu (replicated: full x) ==========
            x_bf = const.tile([P, KC_H], bt)
            for hc in range(KC_H):
                px = ps.tile([P, 1], dt, tag="ps")
                for kc in range(KC_IH):
                    rhs = ain_bf[:, kc:kc + 1] if kc < 8 else uA_bf[:, kc - 8:kc - 7]
                    nc.tensor.matmul(px[:], cwt[:, kc, hc], rhs,
                                     start=(kc == 0), stop=(kc == KC_IH - 1))
                xs = const.tile([P, 1], dt, tag="xs")
                nc.scalar.activation(xs[:], px[:], AF.Relu, bias=cb[:, hc:hc + 1])
                nc.vector.tensor_copy(x_bf[:, hc:hc + 1], xs[:])

            # ================= Stage C: LSTM step =================
            bias2 = const.tile([P, 4], dt)
            nc.vector.tensor_add(bias2[:], bih, bhh)
            gates = []
            for g in range(4):
                pg = ps.tile([P, 1], dt, tag="ps")
                for kc in range(KC_IH):
                    rhs = x_bf[:, kc:kc + 1] if kc < 8 else ain_bf[:, kc:kc + 1]
                    nc.tensor.matmul(pg[:], gwt[:, g, kc], rhs,
                                     start=(kc == 0), stop=(kc == KC_IH - 1))
                act = AF.Tanh if g == 2 else AF.Sigmoid
                gs = const.tile([P, 1], dt, tag=f"gate{g}")
                nc.scalar.activation(gs[:], pg[:], act, bias=bias2[:, g:g + 1])
                gates.append(gs)
            i_g, f_g, g_g, o_g = gates
            t1 = const.tile([P, 1], dt)
            nc.vector.tensor_mul(t1[:], f_g[:], c0)
            t2 = const.tile([P, 1], dt)
            nc.vector.tensor_mul(t2[:], i_g[:], g_g[:])
            c1 = const.tile([P, 1], dt)
            nc.vector.tensor_add(c1[:], t1[:], t2[:])
            tc1 = const.tile([P, 1], dt)
            nc.scalar.activation(tc1[:], c1[:], AF.Tanh)
            h1 = const.tile([P, 1], dt)
            nc.vector.tensor_mul(h1[:], o_g[:], tc1[:])
            nc.gpsimd.dma_start(c_o[:], c1[:])
            nc.gpsimd.dma_start(h_o[:], h1[:])
            h1_bf = const.tile([P, 1], bt)
            nc.vector.tensor_copy(h1_bf[:], h1[:])

            # ==== Stage D: partial logits for the FULL vocab (h-sharded) ====
            logits = const.tile([P, MC_D], dt)
            ow_dmas = []
            OW_PACE = 6   # completion-chained: caps in-flight depth so the
            for g8 in range(N_GRP):   # collective meshes see spare bandwidth
                owt_t = owp.tile([P, D_GRP, P], bt, tag="ow")
                ow_dma = nc.sync.dma_start(
                    owt_t[:], owt_d[:, g8 * D_GRP:(g8 + 1) * D_GRP, :])
                if g8 >= OW_PACE:
                    _add_dep_helper(ow_dma.ins, ow_dmas[g8 - OW_PACE].ins,
                                    reason="pace ow stream")
                ow_dmas.append(ow_dma)
                pd = psd.tile([P, D_GRP], dt, tag="pd")
                for j in range(D_GRP):
                    nc.tensor.matmul(pd[:, j:j + 1], owt_t[:, j], h1_bf[:],
                                     start=True, stop=True)
                nc.vector.tensor_copy(
                    logits[:, g8 * D_GRP:(g8 + 1) * D_GRP], pd[:])

            # CC2: AllReduce partial logits (p-major payload, 205KB)
            nc.gpsimd.dma_start(ar_i.rearrange("(p mc) -> p mc", p=P), logits[:])
            nc.gpsimd.collective_compute(
                "AllReduce", mybir.AluOpType.add, replica_groups=RG,
                ins=[ar_i.ap().opt()], outs=[ar_o.ap().opt()])
            lf_bf = const.tile([P, MC_D], bt)
            nc.sync.dma_start(lf_bf[:], ar_o.rearrange("(p mc) -> p mc", p=P))
            lfull = const.tile([P, MC_D], dt)
            nc.vector.tensor_add(lfull[:], lf_bf[:], ob)

            # local log-softmax over the full padded vocab
            exps = const.tile([P, MC_D], dt)
            rowsum = const.tile([P, 1], dt)
            nc.scalar.activation(exps[:], lfull[:], AF.Exp,
                                 accum_out=rowsum[:])
            psv = ps.tile([P, 1], dt, tag="ps")
            nc.tensor.matmul(psv[:], ones128[:], rowsum[:])
            sv128 = const.tile([P, 1], dt)
            nc.vector.tensor_copy(sv128[:], psv[:])
            logzb = const.tile([P, 1], dt)
            nc.scalar.activation(logzb[:], sv128[:], AF.Ln)

            # full logp out (p-major); the host slices each core's range
            logp = const.tile([P, MC_D], dt)
            nc.vector.tensor_scalar_sub(logp[:], lfull[:], logzb[:])
            nc.sync.dma_start(logp_o.rearrange("(p mc) -> p mc", p=P), logp[:])

    nc.compile()
    return nc


def _prep_inputs(x):
    """Host-side shard/transpose/tile. Pure data movement + one row gather."""
    tok = int(np.asarray(x["input_token"]).reshape(-1)[0])
    emb_row = np.asarray(x["emb"])[tok].astype(F32)          # [1024]
    h0 = np.asarray(x["h"], F32).reshape(H)
    c0 = np.asarray(x["c"], F32).reshape(H)
    attn_in = np.concatenate([emb_row, h0])                   # [2048]
    attn_W = np.asarray(x["attn_W"], F32)
    attn_b = np.asarray(x["attn_b"], F32)
    enc = np.asarray(x["encoder_outputs"], F32)
    comb_W = np.asarray(x["comb_W"], F32)
    comb_b = np.asarray(x["comb_b"], F32)
    Wcat = np.concatenate([np.asarray(x["W_ih"], F32),
                           np.asarray(x["W_hh"], F32)], axis=1)  # [4096, 2048]
    b_ih = np.asarray(x["b_ih"], F32).reshape(4, NCORES, P)
    b_hh = np.asarray(x["b_hh"], F32).reshape(4, NCORES, P)
    out_W = np.asarray(x["out_W"], F32)
    out_b = np.asarray(x["out_b"], F32)
    ow_pad = np.zeros((V_PAD, H), F32)
    ow_pad[:V] = out_W
    ob_pad = np.full((V_PAD,), -1e30, F32)
    ob_pad[:V] = out_b

    ain_c = np.ascontiguousarray(attn_in.reshape(16, P).T)    # [128,16]
    # comb replicated: [j, kc, hc, p] = comb_W[hc*128+p, kc*128+j]
    cwt = np.ascontiguousarray(
        comb_W.reshape(KC_H, P, KC_IH, P).transpose(3, 2, 0, 1).astype(BF16))

    in_maps = []
    for k in range(NCORES):
        A_k = attn_W[k * LK:(k + 1) * LK]
        awt = np.ascontiguousarray(
            A_k.reshape(MC_A, P, KC_IH, P).transpose(3, 0, 2, 1).astype(BF16))
        enc_aug = np.concatenate(
            [enc[k * LK:(k + 1) * LK], np.ones((LK, P), F32)], axis=1)
        enc_t = np.ascontiguousarray(
            enc_aug.reshape(MC_A, P, H + P).transpose(1, 0, 2).astype(BF16))
        gwt = np.empty((P, 4, KC_IH, P), BF16)
        for g in range(4):
            G = Wcat[g * H + k * P: g * H + (k + 1) * P]
            gwt[:, g] = G.reshape(P, KC_IH, P).transpose(2, 1, 0).astype(BF16)
        # out_W h-sharded: [j, mc, v] = ow_pad[mc*128+v, k*128+j]
        owt = np.ascontiguousarray(
            ow_pad[:, k * P:(k + 1) * P]
            .reshape(MC_D, P, P).transpose(2, 0, 1).astype(BF16))
        small = np.zeros((P, SMALL_COLS), F32)
        small[:, C_AIN:C_AIN + 16] = ain_c
        small[:, C_AB:C_AB + 4] = attn_b[k * LK:(k + 1) * LK].reshape(MC_A, P).T
        small[:, C_CB:C_CB + 8] = comb_b.reshape(KC_H, P).T
        small[:, C_BIH:C_BIH + 4] = b_ih[:, k, :].T
        small[:, C_BHH:C_BHH + 4] = b_hh[:, k, :].T
        small[:, C_C0] = c0[k * P:(k + 1) * P]
        small[:, C_OB:C_OB + MC_D] = ob_pad.reshape(MC_D, P).T
        in_maps.append({
            "awt": awt, "enc": enc_t, "cwt": cwt, "gwt": gwt,
            "owt": owt, "small": np.ascontiguousarray(small),
        })
    return in_maps


def _install_ntff_hook():
    """Recreate the missing antenv.axon_hooks shim so trace=True works."""
    import types, ctypes, contextlib

    if "antenv.axon_hooks" in sys.modules:
        return
    so_path = "/opt/axon/libaxon_pjrt.so"
    lib = ctypes.CDLL(so_path)
    if not hasattr(lib, "axon_start_nrt_profile"):
        return
    lib.axon_start_nrt_profile.argtypes = [ctypes.POINTER(ctypes.c_int64),
                                           ctypes.c_size_t]
    lib.axon_start_nrt_profile.restype = ctypes.c_int64
    lib.axon_stop_nrt_profile.argtypes = [ctypes.c_char_p]
    lib.axon_stop_nrt_profile.restype = ctypes.c_int64

    @contextlib.contextmanager
    def _hook(output_dir, device_ids):
        import jax
        jax.devices()
        if device_ids:
            ids = (ctypes.c_int64 * len(device_ids))(*device_ids)
            rc = lib.axon_start_nrt_profile(ids, len(device_ids))
        else:
            rc = lib.axon_start_nrt_profile(None, 0)
        if rc != 0:
            raise RuntimeError(f"axon_start_nrt_profile rc={rc}")
        try:
            yield
        finally:
            n = lib.axon_stop_nrt_profile(str(output_dir).encode())
            print(f"ntff profile: {n} file(s) written to {output_dir}",
                  file=sys.stderr)

    mod = types.ModuleType("antenv.axon_hooks")
    mod._hook = _hook
    mod.get_axon_ntff_profile_hook = lambda: _hook
    mod.set_axon_ntff_profile_hook = lambda h: None
    sys.modules["antenv.axon_hooks"] = mod


def _run(in_maps, trace=False, **kw):
    from concourse import bass_utils
    if trace:
        _install_ntff_hook()
    if "nc" not in _CACHE:
        _CACHE["nc"] = _build_program()
    return bass_utils.run_bass_kernel_spmd(
        _CACHE["nc"], in_maps, core_ids=list(range(NCORES)), trace=trace, **kw)


def _logp_slice(res_k, k):
    full = res_k["logp_o"].reshape(P, MC_D).T.reshape(-1)   # vocab order
    return full[k * VK:(k + 1) * VK]


def kernel(**inputs):
    in_maps = _prep_inputs(inputs)
    res = _run(in_maps)
    logp = np.concatenate(
        [_logp_slice(res.results[k], k) for k in range(NCORES)])[:V].reshape(1, V)
    attnw = np.concatenate([r["attnw_o"] for r in res.results]).reshape(1, L)
    h1 = np.concatenate([r["h_o"] for r in res.results]).reshape(1, 1, H)
    c1 = np.concatenate([r["c_o"] for r in res.results]).reshape(1, 1, H)
    return logp, (h1, c1), attnw
